# revision 2
# baseline (speedup 1.0000x reference)
"""ArcMargin softmax-with-loss on 8 TRN2 NeuronCores — v2.

Design (vs v1):
  - Rows HOST-PERMUTED so each core's locally-owned target rows are
    contiguous (loss mean is row-permutation invariant). The per-row
    target value is fetched by ONE indirect DMA gather from DRAM at
    kernel start -> the per-chunk DVE masked-reduce extract is GONE.
  - Input downcast on host to f16 (or fp8e3): 2x/4x less HBM traffic.
  - The rowwise sum of exp(S*x - 30) is split across THREE engines:
      ACT:  exp via table, fused accum (rate (FD+352)/1.2 ns, any dtype)
      DVE:  u16-Schraudolph: u = rint(A*x+B) saturating -> bitcast f16
            -> relu -> fused accum (2 tensor_scalar passes)
      Pool: same u16 trick on gpsimd
    Systematic biases (quantization + Schraudolph sawtooth) are removed
    by constant factors K_ACT / K_SCH calibrated offline at import
    against the exact pipeline semantics (rint+saturate convert).
  - One warmup AllReduce at t=0 absorbs CC-engine init + inter-core
    launch skew under the stream; ONE final AllReduce [128,8] carries
    Z' and the masked target terms.
  - ln(Z) via DVE bit-trick log2 (+ quadratic mantissa correction):
    no second ACT table load in the tail.

loss = mean_rows(ln(Z') + SHIFT - S*phi_target)   (GAMMA=0)
"""

import math

import numpy as np

import concourse.bacc as bacc
import concourse.bass as bass
import concourse.tile as tile
from concourse.tile import add_dep_helper
from concourse import mybir
from concourse import bass_utils

S = 30.0
M = 0.5
COS_M = math.cos(M)
SIN_M = math.sin(M)
TH = math.cos(math.pi - M)
MM = math.sin(math.pi - M) * M
SHIFT = 30.0

N_CORES = 8
B = 512
C = 100000
C_LOC = C // N_CORES  # 12500
P = 128
NBLK = B // P  # 4

F32 = mybir.dt.float32
F16 = mybir.dt.float16
U16 = mybir.dt.uint16
I32 = mybir.dt.int32
F8E3 = mybir.dt.float8e3
AF = mybir.ActivationFunctionType
ALU = mybir.AluOpType

# Schraudolph constants: u16 = rint(A*x + B_T); bitcast f16 ~ exp(S*x-30)
A_T = 1024.0 * S / math.log(2.0)       # 44287.98
B_T = 1024.0 * 15.0 - A_T              # -28927.98  (bias 15)

# per-block column chunking (block-sequential streaming)
BLOCK_SPLITS = [
    [2500, 4000, 6000],
    [6250, 6250],
    [6250, 6250],
    [6000, 4000, 2500],
]
# engine split fractions (ACT, DVE, Pool); remainder of w goes to Pool.
# Pool runs pass1 only (TensorScalar+accum is DVE-only on this ISA); DVE
# runs pass2 for both the DVE and Pool slices.
SPLITS_F16 = (0.83, 0.06)
SPLITS_F8 = (0.47, 0.16)


def _round4(x):
    return int(x) // 4 * 4


def _calibrate(f8: bool):
    """K_ACT, K_SCH: exact-pipeline bias correction on the input dist
    (uniform[-1,1] f32 -> f16/fp8 quantized). Data independent."""
    import ml_dtypes

    g = np.linspace(-1.0, 1.0, 2_000_001, dtype=np.float64).astype(np.float32)
    exact = np.exp(S * g.astype(np.float64) - 30.0)
    if f8:
        q = g.astype(ml_dtypes.float8_e3m4).astype(np.float32)
    else:
        q = g.astype(np.float16).astype(np.float32)
    eq = np.exp(S * q.astype(np.float64) - 30.0)
    k_act = float(exact.sum() / eq.sum())
    t = q * np.float32(A_T) + np.float32(B_T)
    u = np.clip(np.rint(t), 0, 65535).astype(np.uint16)
    tr = np.maximum(u.view(np.float16).astype(np.float64), 0.0)
    k_sch = float(exact.sum() / tr.sum())
    return k_act, k_sch


_CAL = {}


def get_cal(f8):
    if f8 not in _CAL:
        _CAL[f8] = _calibrate(f8)
    return _CAL[f8]


def build(f8=False, splits=None, stream_bufs=7, act_junk_f8=True,
          gather_single=False):
    xdt = F8E3 if f8 else F16
    frac_a, frac_d = splits or (SPLITS_F8 if f8 else SPLITS_F16)
    bs = BLOCK_SPLITS
    assert all(sum(s) == C_LOC for s in bs)
    nch = sum(len(s) for s in bs)
    k_act, k_sch = get_cal(f8)

    nc = bacc.Bacc(
        "TRN2", target_bir_lowering=False, debug=False, num_devices=N_CORES
    )
    x = nc.dram_tensor("x", [B * C_LOC, 1], xdt, kind="ExternalInput")
    goff = nc.dram_tensor("goff", [P, NBLK], I32, kind="ExternalInput")
    maskd = nc.dram_tensor("mask", [P, NBLK], F32, kind="ExternalInput")
    out = nc.dram_tensor("out", [1, 1], F32, kind="ExternalOutput")

    x3 = x.ap().rearrange("(k p c) one -> k p (c one)", k=NBLK, p=P, c=C_LOC)

    with tile.TileContext(nc) as tc:
        with (
            tc.tile_pool(name="stream", bufs=stream_bufs) as stream,
            tc.tile_pool(name="ea", bufs=2) as ea,       # ACT junk out
            tc.tile_pool(name="dup", bufs=2) as dup,     # DVE upconvert (f8)
            tc.tile_pool(name="du16", bufs=2) as du16,   # DVE u16
            tc.tile_pool(name="dj", bufs=2) as dj,       # DVE junk relu out
            tc.tile_pool(name="pu16", bufs=2) as pu16,   # Pool u16
            tc.tile_pool(name="pj", bufs=2) as pj,       # Pool junk relu out
            tc.tile_pool(name="small", bufs=1) as small,
            tc.tile_pool(name="dram", bufs=1, space="DRAM") as dram,
            tc.tile_pool(name="psum", bufs=1, space="PSUM") as psum,
        ):
            # NOTE: no warmup AllReduce — its trigger blocks the gpsimd
            # queue until the CC engine arms (~18us observed) and the CC
            # stream is in-order, so a warmup op only delays the real AR.

            # ---- metadata + target gather (off the stream path) ----
            gi = small.tile([P, NBLK], I32)
            nc.gpsimd.dma_start(out=gi[:], in_=goff.ap())
            mask_sb = small.tile([P, NBLK], F32)
            nc.gpsimd.dma_start(out=mask_sb[:], in_=maskd.ap())
            nbias = small.tile([P, 1], F32)
            nc.vector.memset(nbias[:], -SHIFT)

            xt = small.tile([P, NBLK], xdt)

            def emit_gathers():
                # 512 single-element descriptors: slow (~15us to drain
                # under stream-DMA contention) but far off the critical
                # path as long as the phi chain is pinned late.
                for g in range(NBLK):
                    nc.gpsimd.indirect_dma_start(
                        out=xt[:, g : g + 1],
                        out_offset=None,
                        in_=x.ap(),
                        in_offset=bass.IndirectOffsetOnAxis(
                            ap=gi[:, g : g + 1], axis=0
                        ),
                    )

            # ---- accumulator columns ----
            zA = small.tile([P, nch], F32)
            zD = small.tile([P, nch], F32)
            zP = small.tile([P, nch], F32)

            # ---- streaming: per chunk, 3-engine split ----
            stream_acts = []
            stream_dves = []
            ci = 0
            for k in range(NBLK):
                off = 0
                for w in bs[k]:
                    wa = _round4(w * frac_a)
                    wd = _round4(w * frac_d)
                    wp = w - wa - wd
                    t = stream.tile([P, w], xdt, tag="stream")
                    nc.sync.dma_start(out=t[:], in_=x3[k, :, off : off + w])
                    off += w
                    # ACT slice
                    e8 = ea.tile([P, wa], F8E3 if act_junk_f8 else F16, tag="ea")
                    act_i = nc.scalar.activation(
                        e8[:], t[:, :wa], AF.Exp, bias=nbias[:], scale=S,
                        accum_out=zA[:, ci : ci + 1],
                    )
                    stream_acts.append(act_i)
                    # DVE slice
                    if f8:
                        uf = dup.tile([P, wd], F16, tag="dup")
                        nc.vector.tensor_scalar(
                            uf[:], t[:, wa : wa + wd], 1.0, None, ALU.mult
                        )
                        dsrc = uf[:]
                    else:
                        dsrc = t[:, wa : wa + wd]
                    ud = du16.tile([P, wd], U16, tag="du16")
                    nc.vector.tensor_scalar(
                        ud[:], dsrc, A_T, B_T, ALU.mult, ALU.add
                    )
                    jd = dj.tile([P, wd], F16, tag="dj")
                    nc.vector.tensor_scalar(
                        jd[:], ud[:].bitcast(F16), 0.0, 0.0, ALU.max, ALU.add,
                        accum_out=zD[:, ci : ci + 1],
                    )
                    # Pool slice: pass1 on Pool, pass2 on DVE (accum is
                    # DVE-only)
                    up = pu16.tile([P, wp], U16, tag="pu16")
                    nc.gpsimd.tensor_scalar(
                        up[:], t[:, wa + wd :], A_T, B_T, ALU.mult, ALU.add
                    )
                    jp = pj.tile([P, wp], F16, tag="pj")
                    dve_i = nc.vector.tensor_scalar(
                        jp[:], up[:].bitcast(F16), 0.0, 0.0, ALU.max, ALU.add,
                        accum_out=zP[:, ci : ci + 1],
                    )
                    stream_dves.append(dve_i)
                    ci += 1
                    if ci == 2:
                        # gathers after the first two chunks' pool ops so
                        # they don't delay the early pipeline
                        emit_gathers()
            assert ci == nch

            # ---- epilogue ----
            col0 = 0
            zz = small.tile([P, NBLK], F32)
            zs = small.tile([P, NBLK], F32)
            t1 = small.tile([P, NBLK], F32)
            for k in range(NBLK):
                n = len(bs[k])
                cs = slice(col0, col0 + n)
                c = slice(k, k + 1)
                nc.vector.tensor_reduce(
                    zz[:, c], zA[:, cs], axis=mybir.AxisListType.X, op=ALU.add
                )
                nc.vector.tensor_reduce(
                    zs[:, c], zD[:, cs], axis=mybir.AxisListType.X, op=ALU.add
                )
                nc.vector.tensor_reduce(
                    t1[:, c], zP[:, cs], axis=mybir.AxisListType.X, op=ALU.add
                )
                col0 += n
            # z = k_act*zz + k_sch*(zs + t1)
            nc.vector.tensor_add(zs[:], zs[:], t1[:])
            nc.vector.tensor_scalar(zz[:], zz[:], k_act, None, ALU.mult)
            nc.vector.tensor_scalar(zs[:], zs[:], k_sch, None, ALU.mult)
            nc.vector.tensor_add(zz[:], zz[:], zs[:])

            # phi on xt: sin = sqrt(relu(1-x^2)) via Newton rsqrt (DVE only)
            # Pinned behind chunk-6's DVE op: the slow gathers feed xt, and
            # an early-hoisted phi chain stalls the in-order DVE queue
            # (observed: 9us stream hole). Late-middle placement runs it in
            # DVE slack without delaying the stream or the tail.
            xg = small.tile([P, NBLK], F32)
            cast_i = nc.vector.tensor_copy(xg[:], xt[:])
            add_dep_helper(cast_i.ins, stream_dves[6].ins, sync=False,
                           reason="phi chain after mid-stream")
            s2 = small.tile([P, NBLK], F32)
            sh = small.tile([P, NBLK], mybir.dt.uint32)
            r_u = small.tile([P, NBLK], mybir.dt.uint32)
            tt = small.tile([P, NBLK], F32)
            phi = small.tile([P, NBLK], F32)
            alt = small.tile([P, NBLK], F32)
            cond = small.tile([P, NBLK], I32)
            nc.vector.tensor_mul(s2[:], xg[:], xg[:])
            nc.vector.tensor_scalar(s2[:], s2[:], -1.0, 1.0, ALU.mult, ALU.add)
            # clamp strictly above 0: quantized x can hit exactly +-1 and
            # the rsqrt seed overflows f32 on s2=0
            nc.vector.tensor_scalar_max(s2[:], s2[:], 1e-12)
            nc.vector.tensor_scalar(
                sh[:], s2[:].bitcast(mybir.dt.uint32), 1, None,
                ALU.logical_shift_right,
            )
            nc.vector.tensor_scalar(
                r_u[:], sh[:], -1.0, float(0x5F3759DF), ALU.mult, ALU.add
            )
            r = r_u[:].bitcast(F32)
            for _ in range(2):
                nc.vector.tensor_mul(tt[:], r, r)
                nc.vector.tensor_mul(tt[:], tt[:], s2[:])
                nc.vector.tensor_scalar(tt[:], tt[:], -0.5, 1.5, ALU.mult, ALU.add)
                nc.vector.tensor_mul(r, r, tt[:])
            nc.vector.tensor_mul(tt[:], r, s2[:])  # sin
            nc.vector.tensor_scalar(tt[:], tt[:], SIN_M, None, ALU.mult)
            nc.vector.tensor_scalar(phi[:], xg[:], COS_M, None, ALU.mult)
            nc.vector.tensor_sub(phi[:], phi[:], tt[:])
            nc.vector.tensor_scalar(alt[:], xg[:], -MM, None, ALU.add)
            nc.vector.tensor_scalar(cond[:], xg[:], TH, None, ALU.is_le)
            nc.vector.copy_predicated(phi[:], cond[:], alt[:])
            # corr = (exp(S*phi-30) - exp(S*xg-30)) * mask ; tgt = S*phi*mask
            e1 = small.tile([P, NBLK], F32)
            e2 = small.tile([P, NBLK], F32)
            e1i = nc.scalar.activation(e1[:], phi[:], AF.Exp, bias=nbias[:],
                                       scale=S)
            e2i = nc.scalar.activation(e2[:], xg[:], AF.Exp, bias=nbias[:],
                                       scale=S)
            # The ACT queue is in-order: without explicit edges the Tile
            # scheduler hoists these tiny exps (which wait on the phi
            # chain) ahead of later stream ACTIVATEs, stalling the whole
            # queue ~15us. Pin them behind the last stream chunk.
            for ei in (e1i, e2i):
                add_dep_helper(ei.ins, stream_acts[-1].ins, sync=False,
                               reason="epilogue exps after stream")
            nc.vector.tensor_sub(e1[:], e1[:], e2[:])
            nc.vector.tensor_mul(e1[:], e1[:], mask_sb[:])
            ar_in = small.tile([P, 2 * NBLK], F32)
            nc.vector.tensor_add(ar_in[:, 0:NBLK], zz[:], e1[:])
            tgt = ar_in[:, NBLK : 2 * NBLK]
            nc.vector.tensor_scalar(tgt, phi[:], S, None, ALU.mult)
            nc.vector.tensor_mul(tgt, tgt, mask_sb[:])

            # ---- final AllReduce ----
            cc_in = dram.tile([P, 2 * NBLK], F32)
            cc_out = dram.tile([P, 2 * NBLK], F32)
            nc.gpsimd.dma_start(out=cc_in[:], in_=ar_in[:])
            nc.gpsimd.collective_compute(
                "AllReduce",
                ALU.add,
                replica_groups=[list(range(N_CORES))],
                ins=[cc_in.opt()],
                outs=[cc_out.opt()],
            )
            g = small.tile([P, 2 * NBLK], F32)
            nc.gpsimd.dma_start(out=g[:], in_=cc_out[:])

            # ---- ln via DVE bit-trick log2 + quadratic correction ----
            zb = g[:, 0:NBLK].bitcast(I32)
            l1 = small.tile([P, NBLK], F32)
            mb = small.tile([P, NBLK], I32)
            f = small.tile([P, NBLK], F32)
            f2 = small.tile([P, NBLK], F32)
            lnz = small.tile([P, NBLK], F32)
            nc.vector.tensor_scalar(
                l1[:], zb, 2.0 ** -23, -127.0, ALU.mult, ALU.add
            )
            nc.vector.tensor_scalar(mb[:], zb, 0x7FFFFF, None, ALU.bitwise_and)
            nc.vector.tensor_scalar(f[:], mb[:], 2.0 ** -23, None, ALU.mult)
            nc.vector.tensor_scalar(f2[:], f[:], -1.0, 1.0, ALU.mult, ALU.add)
            nc.vector.tensor_mul(f2[:], f2[:], f[:])
            # log2(1+f) >= f: l1 = e + f underestimates, so ADD the
            # quadratic correction:  ln z = ln2*(l1 + 0.3431*f*(1-f))
            nc.vector.tensor_scalar(l1[:], l1[:], math.log(2.0), None, ALU.mult)
            nc.vector.tensor_scalar(
                f2[:], f2[:], math.log(2.0) * 0.3431, None, ALU.mult
            )
            nc.vector.tensor_add(lnz[:], l1[:], f2[:])
            # t = lnz - tgt ; loss = sum(t)/B + SHIFT
            nc.vector.tensor_sub(lnz[:], lnz[:], g[:, NBLK : 2 * NBLK])
            r1 = small.tile([P, 1], F32)
            nc.vector.tensor_reduce(
                r1[:], lnz[:], axis=mybir.AxisListType.X, op=ALU.add
            )
            ones = small.tile([P, 1], F32)
            nc.vector.memset(ones[:], 1.0)
            ps = psum.tile([1, 1], F32)
            nc.tensor.matmul(ps[:], lhsT=r1[:], rhs=ones[:], start=True, stop=True)
            loss = small.tile([1, 1], F32)
            nc.vector.tensor_scalar(
                loss[:], ps[:], 1.0 / B, SHIFT, ALU.mult, ALU.add
            )
            nc.sync.dma_start(out=out.ap(), in_=loss[:])
    nc.finalize()
    return nc


def prep_in_maps(cos_theta, target, f8=False):
    import ml_dtypes

    cos_theta = np.ascontiguousarray(np.asarray(cos_theta), dtype=np.float32)
    target = np.asarray(target).astype(np.int64)
    owner = (target // C_LOC).astype(np.int64)
    perm = np.argsort(owner, kind="stable")
    xp = cos_theta[perm]
    tp = target[perm]
    op_ = owner[perm]
    if f8:
        xq = xp.astype(ml_dtypes.float8_e3m4).view(np.uint8)
    else:
        xq = xp.astype(np.float16)
    rows = np.arange(B, dtype=np.int64)
    in_maps = []
    for i in range(N_CORES):
        lo = i * C_LOC
        sh = np.ascontiguousarray(xq[:, lo : lo + C_LOC]).reshape(B * C_LOC, 1)
        own = op_ == i
        off = np.where(own, rows * C_LOC + (tp - lo), 0).astype(np.int32)
        goff = off.reshape(NBLK, P).T.copy()  # [P, NBLK]
        msk = own.astype(np.float32).reshape(NBLK, P).T.copy()
        in_maps.append({"x": sh, "goff": goff, "mask": msk})
    return in_maps


_CACHE = {}


def _get_nc(key, **kw):
    if key not in _CACHE:
        _CACHE[key] = build(**kw)
    return _CACHE[key]


def run(cos_theta, target, trace=False, f8=False, **bkw):
    nc = _get_nc(("nc", f8, tuple(sorted(bkw.items()))), f8=f8, **bkw)
    in_maps = prep_in_maps(cos_theta, target, f8=f8)
    res = bass_utils.run_bass_kernel_spmd(
        nc, in_maps, core_ids=list(range(N_CORES)), trace=trace
    )
    loss = np.asarray(res.results[0]["out"], dtype=np.float32).reshape(())
    return loss, res.exec_time_ns


def kernel(cos_theta, target):
    loss, _ = run(cos_theta, target)
    return loss


# revision 3
# speedup vs baseline: 1.0831x; 1.0831x over previous
"""ArcMargin softmax-with-loss on 8 TRN2 NeuronCores — v2.

Design (vs v1):
  - Rows HOST-PERMUTED so each core's locally-owned target rows are
    contiguous (loss mean is row-permutation invariant). The per-row
    target value is fetched by ONE indirect DMA gather from DRAM at
    kernel start -> the per-chunk DVE masked-reduce extract is GONE.
  - Input downcast on host to f16 (or fp8e3): 2x/4x less HBM traffic.
  - The rowwise sum of exp(S*x - 30) is split across THREE engines:
      ACT:  exp via table, fused accum (rate (FD+352)/1.2 ns, any dtype)
      DVE:  u16-Schraudolph: u = rint(A*x+B) saturating -> bitcast f16
            -> relu -> fused accum (2 tensor_scalar passes)
      Pool: same u16 trick on gpsimd
    Systematic biases (quantization + Schraudolph sawtooth) are removed
    by constant factors K_ACT / K_SCH calibrated offline at import
    against the exact pipeline semantics (rint+saturate convert).
  - One warmup AllReduce at t=0 absorbs CC-engine init + inter-core
    launch skew under the stream; ONE final AllReduce [128,8] carries
    Z' and the masked target terms.
  - ln(Z) via DVE bit-trick log2 (+ quadratic mantissa correction):
    no second ACT table load in the tail.

loss = mean_rows(ln(Z') + SHIFT - S*phi_target)   (GAMMA=0)
"""

import math

import numpy as np

import concourse.bacc as bacc
import concourse.bass as bass
import concourse.tile as tile
from concourse.tile import add_dep_helper
from concourse import mybir
from concourse import bass_utils

S = 30.0
M = 0.5
COS_M = math.cos(M)
SIN_M = math.sin(M)
TH = math.cos(math.pi - M)
MM = math.sin(math.pi - M) * M
SHIFT = 30.0

N_CORES = 8
B = 512
C = 100000
C_LOC = C // N_CORES  # 12500
P = 128
NBLK = B // P  # 4

F32 = mybir.dt.float32
F16 = mybir.dt.float16
U16 = mybir.dt.uint16
I32 = mybir.dt.int32
F8E3 = mybir.dt.float8e3
AF = mybir.ActivationFunctionType
ALU = mybir.AluOpType

# Schraudolph constants: u16 = rint(A*x + B_T); bitcast f16 ~ exp(S*x-30)
A_T = 1024.0 * S / math.log(2.0)       # 44287.98
B_T = 1024.0 * 15.0 - A_T              # -28927.98  (bias 15)

# per-block column chunking (block-sequential streaming)
BLOCK_SPLITS = [
    [2500, 4000, 6000],
    [6250, 6250],
    [6250, 6250],
    [6000, 4000, 2500],
]
# engine split fractions (ACT, DVE, Pool); remainder of w goes to Pool.
# Pool runs pass1 only (TensorScalar+accum is DVE-only on this ISA); DVE
# runs pass2 for both the DVE and Pool slices.
SPLITS_F16 = (0.83, 0.06)
SPLITS_F8 = (0.47, 0.16)


def _round4(x):
    return int(x) // 4 * 4


def _calibrate(f8: bool):
    """K_ACT, K_SCH: exact-pipeline bias correction on the input dist
    (uniform[-1,1] f32 -> f16/fp8 quantized). Data independent."""
    import ml_dtypes

    g = np.linspace(-1.0, 1.0, 2_000_001, dtype=np.float64).astype(np.float32)
    exact = np.exp(S * g.astype(np.float64) - 30.0)
    if f8:
        q = g.astype(ml_dtypes.float8_e3m4).astype(np.float32)
    else:
        q = g.astype(np.float16).astype(np.float32)
    eq = np.exp(S * q.astype(np.float64) - 30.0)
    k_act = float(exact.sum() / eq.sum())
    t = q * np.float32(A_T) + np.float32(B_T)
    u = np.clip(np.rint(t), 0, 65535).astype(np.uint16)
    tr = np.maximum(u.view(np.float16).astype(np.float64), 0.0)
    k_sch = float(exact.sum() / tr.sum())
    return k_act, k_sch


_CAL = {}


def get_cal(f8):
    if f8 not in _CAL:
        _CAL[f8] = _calibrate(f8)
    return _CAL[f8]


def build(f8=False, splits=None, stream_bufs=7, act_junk_f8=True,
          gather_single=False):
    xdt = F8E3 if f8 else F16
    frac_a, frac_d = splits or (SPLITS_F8 if f8 else SPLITS_F16)
    bs = BLOCK_SPLITS
    assert all(sum(s) == C_LOC for s in bs)
    nch = sum(len(s) for s in bs)
    k_act, k_sch = get_cal(f8)

    nc = bacc.Bacc(
        "TRN2", target_bir_lowering=False, debug=False, num_devices=N_CORES
    )
    x = nc.dram_tensor("x", [B * C_LOC, 1], xdt, kind="ExternalInput")
    goff = nc.dram_tensor("goff", [P, NBLK], I32, kind="ExternalInput")
    maskd = nc.dram_tensor("mask", [P, NBLK], F32, kind="ExternalInput")
    out = nc.dram_tensor("out", [1, 1], F32, kind="ExternalOutput")

    x3 = x.ap().rearrange("(k p c) one -> k p (c one)", k=NBLK, p=P, c=C_LOC)

    with tile.TileContext(nc) as tc:
        with (
            tc.tile_pool(name="stream", bufs=stream_bufs) as stream,
            tc.tile_pool(name="ea", bufs=2) as ea,       # ACT junk out
            tc.tile_pool(name="dup", bufs=2) as dup,     # DVE upconvert (f8)
            tc.tile_pool(name="du16", bufs=2) as du16,   # DVE u16
            tc.tile_pool(name="dj", bufs=2) as dj,       # DVE junk relu out
            tc.tile_pool(name="pu16", bufs=2) as pu16,   # Pool u16
            tc.tile_pool(name="pj", bufs=2) as pj,       # Pool junk relu out
            tc.tile_pool(name="small", bufs=1) as small,
            tc.tile_pool(name="dram", bufs=1, space="DRAM") as dram,
            tc.tile_pool(name="psum", bufs=1, space="PSUM") as psum,
        ):
            # NOTE: no warmup AllReduce — its trigger blocks the gpsimd
            # queue until the CC engine arms (~18us observed) and the CC
            # stream is in-order, so a warmup op only delays the real AR.

            # ---- metadata + target gather (off the stream path) ----
            gi = small.tile([P, NBLK], I32)
            nc.gpsimd.dma_start(out=gi[:], in_=goff.ap())
            mask_sb = small.tile([P, NBLK], F32)
            nc.gpsimd.dma_start(out=mask_sb[:], in_=maskd.ap())
            nbias = small.tile([P, 1], F32)
            nc.vector.memset(nbias[:], -SHIFT)

            xt = small.tile([P, NBLK], xdt)

            def emit_gathers():
                # 512 single-element descriptors: slow (~15us to drain
                # under stream-DMA contention) but far off the critical
                # path as long as the phi chain is pinned late.
                for g in range(NBLK):
                    nc.gpsimd.indirect_dma_start(
                        out=xt[:, g : g + 1],
                        out_offset=None,
                        in_=x.ap(),
                        in_offset=bass.IndirectOffsetOnAxis(
                            ap=gi[:, g : g + 1], axis=0
                        ),
                    )

            # ---- accumulator columns ----
            zA = small.tile([P, nch], F32)
            zD = small.tile([P, nch], F32)
            zP = small.tile([P, nch], F32)

            # ---- streaming: per chunk, 3-engine split ----
            stream_acts = []
            stream_dves = []
            ci = 0
            for k in range(NBLK):
                off = 0
                for w in bs[k]:
                    wa = _round4(w * frac_a)
                    wd = _round4(w * frac_d)
                    wp = w - wa - wd
                    t = stream.tile([P, w], xdt, tag="stream")
                    nc.sync.dma_start(out=t[:], in_=x3[k, :, off : off + w])
                    off += w
                    # ACT slice
                    e8 = ea.tile([P, wa], F8E3 if act_junk_f8 else F16, tag="ea")
                    act_i = nc.scalar.activation(
                        e8[:], t[:, :wa], AF.Exp, bias=nbias[:], scale=S,
                        accum_out=zA[:, ci : ci + 1],
                    )
                    stream_acts.append(act_i)
                    # DVE slice
                    if f8:
                        uf = dup.tile([P, wd], F16, tag="dup")
                        nc.vector.tensor_scalar(
                            uf[:], t[:, wa : wa + wd], 1.0, None, ALU.mult
                        )
                        dsrc = uf[:]
                    else:
                        dsrc = t[:, wa : wa + wd]
                    ud = du16.tile([P, wd], U16, tag="du16")
                    nc.vector.tensor_scalar(
                        ud[:], dsrc, A_T, B_T, ALU.mult, ALU.add
                    )
                    jd = dj.tile([P, wd], F16, tag="dj")
                    nc.vector.tensor_scalar(
                        jd[:], ud[:].bitcast(F16), 0.0, 0.0, ALU.max, ALU.add,
                        accum_out=zD[:, ci : ci + 1],
                    )
                    # Pool slice: pass1 on Pool, pass2 on DVE (accum is
                    # DVE-only)
                    up = pu16.tile([P, wp], U16, tag="pu16")
                    nc.gpsimd.tensor_scalar(
                        up[:], t[:, wa + wd :], A_T, B_T, ALU.mult, ALU.add
                    )
                    jp = pj.tile([P, wp], F16, tag="pj")
                    dve_i = nc.vector.tensor_scalar(
                        jp[:], up[:].bitcast(F16), 0.0, 0.0, ALU.max, ALU.add,
                        accum_out=zP[:, ci : ci + 1],
                    )
                    stream_dves.append(dve_i)
                    ci += 1
                    if ci == 2:
                        # gathers after the first two chunks' pool ops so
                        # they don't delay the early pipeline
                        emit_gathers()
            assert ci == nch

            # ---- epilogue ----
            col0 = 0
            zz = small.tile([P, NBLK], F32)
            zs = small.tile([P, NBLK], F32)
            t1 = small.tile([P, NBLK], F32)
            for k in range(NBLK):
                n = len(bs[k])
                cs = slice(col0, col0 + n)
                c = slice(k, k + 1)
                nc.vector.tensor_reduce(
                    zz[:, c], zA[:, cs], axis=mybir.AxisListType.X, op=ALU.add
                )
                nc.vector.tensor_reduce(
                    zs[:, c], zD[:, cs], axis=mybir.AxisListType.X, op=ALU.add
                )
                nc.vector.tensor_reduce(
                    t1[:, c], zP[:, cs], axis=mybir.AxisListType.X, op=ALU.add
                )
                col0 += n
            # z = k_act*zz + k_sch*(zs + t1)
            nc.vector.tensor_add(zs[:], zs[:], t1[:])
            nc.vector.tensor_scalar(zz[:], zz[:], k_act, None, ALU.mult)
            nc.vector.tensor_scalar(zs[:], zs[:], k_sch, None, ALU.mult)
            nc.vector.tensor_add(zz[:], zz[:], zs[:])

            # phi on xt: sin = sqrt(relu(1-x^2)) via Newton rsqrt (DVE only)
            # Pinned behind chunk-6's DVE op: the slow gathers feed xt, and
            # an early-hoisted phi chain stalls the in-order DVE queue
            # (observed: 9us stream hole). Late-middle placement runs it in
            # DVE slack without delaying the stream or the tail.
            xg = small.tile([P, NBLK], F32)
            cast_i = nc.vector.tensor_copy(xg[:], xt[:])
            add_dep_helper(cast_i.ins, stream_dves[5].ins, sync=False,
                           reason="phi chain after mid-stream")
            s2 = small.tile([P, NBLK], F32)
            sh = small.tile([P, NBLK], mybir.dt.uint32)
            r_u = small.tile([P, NBLK], mybir.dt.uint32)
            tt = small.tile([P, NBLK], F32)
            phi = small.tile([P, NBLK], F32)
            alt = small.tile([P, NBLK], F32)
            cond = small.tile([P, NBLK], I32)
            nc.vector.tensor_mul(s2[:], xg[:], xg[:])
            nc.vector.tensor_scalar(s2[:], s2[:], -1.0, 1.0, ALU.mult, ALU.add)
            # clamp strictly above 0: quantized x can hit exactly +-1 and
            # the rsqrt seed overflows f32 on s2=0
            nc.vector.tensor_scalar_max(s2[:], s2[:], 1e-12)
            nc.vector.tensor_scalar(
                sh[:], s2[:].bitcast(mybir.dt.uint32), 1, None,
                ALU.logical_shift_right,
            )
            nc.vector.tensor_scalar(
                r_u[:], sh[:], -1.0, float(0x5F3759DF), ALU.mult, ALU.add
            )
            r = r_u[:].bitcast(F32)
            for _ in range(2):
                nc.vector.tensor_mul(tt[:], r, r)
                nc.vector.tensor_mul(tt[:], tt[:], s2[:])
                nc.vector.tensor_scalar(tt[:], tt[:], -0.5, 1.5, ALU.mult, ALU.add)
                nc.vector.tensor_mul(r, r, tt[:])
            nc.vector.tensor_mul(tt[:], r, s2[:])  # sin
            nc.vector.tensor_scalar(tt[:], tt[:], SIN_M, None, ALU.mult)
            nc.vector.tensor_scalar(phi[:], xg[:], COS_M, None, ALU.mult)
            nc.vector.tensor_sub(phi[:], phi[:], tt[:])
            nc.vector.tensor_scalar(alt[:], xg[:], -MM, None, ALU.add)
            nc.vector.tensor_scalar(cond[:], xg[:], TH, None, ALU.is_le)
            nc.vector.copy_predicated(phi[:], cond[:], alt[:])
            # corr = (exp(S*phi-30) - exp(S*xg-30)) * mask ; tgt = S*phi*mask
            e1 = small.tile([P, NBLK], F32)
            e2 = small.tile([P, NBLK], F32)
            e1i = nc.scalar.activation(e1[:], phi[:], AF.Exp, bias=nbias[:],
                                       scale=S)
            e2i = nc.scalar.activation(e2[:], xg[:], AF.Exp, bias=nbias[:],
                                       scale=S)
            # The ACT queue is in-order: without explicit edges the Tile
            # scheduler hoists these tiny exps (which wait on the phi
            # chain) ahead of later stream ACTIVATEs, stalling the whole
            # queue ~15us. Pin them behind the last stream chunk.
            for ei in (e1i, e2i):
                add_dep_helper(ei.ins, stream_acts[-1].ins, sync=False,
                               reason="epilogue exps after stream")
            nc.vector.tensor_sub(e1[:], e1[:], e2[:])
            nc.vector.tensor_mul(e1[:], e1[:], mask_sb[:])
            ar_in = small.tile([P, 2 * NBLK], F32)
            nc.vector.tensor_add(ar_in[:, 0:NBLK], zz[:], e1[:])
            tgt = ar_in[:, NBLK : 2 * NBLK]
            nc.vector.tensor_scalar(tgt, phi[:], S, None, ALU.mult)
            nc.vector.tensor_mul(tgt, tgt, mask_sb[:])

            # ---- final AllReduce ----
            cc_in = dram.tile([P, 2 * NBLK], F32)
            cc_out = dram.tile([P, 2 * NBLK], F32)
            nc.gpsimd.dma_start(out=cc_in[:], in_=ar_in[:])
            nc.gpsimd.collective_compute(
                "AllReduce",
                ALU.add,
                replica_groups=[list(range(N_CORES))],
                ins=[cc_in.opt()],
                outs=[cc_out.opt()],
            )
            g = small.tile([P, 2 * NBLK], F32)
            nc.gpsimd.dma_start(out=g[:], in_=cc_out[:])

            # ---- ln via DVE bit-trick log2 + quadratic correction ----
            zb = g[:, 0:NBLK].bitcast(I32)
            l1 = small.tile([P, NBLK], F32)
            mb = small.tile([P, NBLK], I32)
            f = small.tile([P, NBLK], F32)
            f2 = small.tile([P, NBLK], F32)
            lnz = small.tile([P, NBLK], F32)
            nc.vector.tensor_scalar(
                l1[:], zb, 2.0 ** -23, -127.0, ALU.mult, ALU.add
            )
            nc.vector.tensor_scalar(mb[:], zb, 0x7FFFFF, None, ALU.bitwise_and)
            nc.vector.tensor_scalar(f[:], mb[:], 2.0 ** -23, None, ALU.mult)
            nc.vector.tensor_scalar(f2[:], f[:], -1.0, 1.0, ALU.mult, ALU.add)
            nc.vector.tensor_mul(f2[:], f2[:], f[:])
            # log2(1+f) >= f: l1 = e + f underestimates, so ADD the
            # quadratic correction:  ln z = ln2*(l1 + 0.3431*f*(1-f))
            nc.vector.tensor_scalar(l1[:], l1[:], math.log(2.0), None, ALU.mult)
            nc.vector.tensor_scalar(
                f2[:], f2[:], math.log(2.0) * 0.3431, None, ALU.mult
            )
            nc.vector.tensor_add(lnz[:], l1[:], f2[:])
            # t = lnz - tgt ; loss = sum(t)/B + SHIFT
            nc.vector.tensor_sub(lnz[:], lnz[:], g[:, NBLK : 2 * NBLK])
            r1 = small.tile([P, 1], F32)
            nc.vector.tensor_reduce(
                r1[:], lnz[:], axis=mybir.AxisListType.X, op=ALU.add
            )
            ones = small.tile([P, 1], F32)
            nc.vector.memset(ones[:], 1.0)
            ps = psum.tile([1, 1], F32)
            nc.tensor.matmul(ps[:], lhsT=r1[:], rhs=ones[:], start=True, stop=True)
            loss = small.tile([1, 1], F32)
            nc.vector.tensor_scalar(
                loss[:], ps[:], 1.0 / B, SHIFT, ALU.mult, ALU.add
            )
            nc.sync.dma_start(out=out.ap(), in_=loss[:])
    nc.finalize()
    return nc


def prep_in_maps(cos_theta, target, f8=False):
    import ml_dtypes

    cos_theta = np.ascontiguousarray(np.asarray(cos_theta), dtype=np.float32)
    target = np.asarray(target).astype(np.int64)
    owner = (target // C_LOC).astype(np.int64)
    perm = np.argsort(owner, kind="stable")
    xp = cos_theta[perm]
    tp = target[perm]
    op_ = owner[perm]
    if f8:
        xq = xp.astype(ml_dtypes.float8_e3m4).view(np.uint8)
    else:
        xq = xp.astype(np.float16)
    rows = np.arange(B, dtype=np.int64)
    in_maps = []
    for i in range(N_CORES):
        lo = i * C_LOC
        sh = np.ascontiguousarray(xq[:, lo : lo + C_LOC]).reshape(B * C_LOC, 1)
        own = op_ == i
        off = np.where(own, rows * C_LOC + (tp - lo), 0).astype(np.int32)
        goff = off.reshape(NBLK, P).T.copy()  # [P, NBLK]
        msk = own.astype(np.float32).reshape(NBLK, P).T.copy()
        in_maps.append({"x": sh, "goff": goff, "mask": msk})
    return in_maps


_CACHE = {}


def _get_nc(key, **kw):
    if key not in _CACHE:
        _CACHE[key] = build(**kw)
    return _CACHE[key]


def run(cos_theta, target, trace=False, f8=False, **bkw):
    nc = _get_nc(("nc", f8, tuple(sorted(bkw.items()))), f8=f8, **bkw)
    in_maps = prep_in_maps(cos_theta, target, f8=f8)
    res = bass_utils.run_bass_kernel_spmd(
        nc, in_maps, core_ids=list(range(N_CORES)), trace=trace
    )
    loss = np.asarray(res.results[0]["out"], dtype=np.float32).reshape(())
    return loss, res.exec_time_ns


def kernel(cos_theta, target):
    loss, _ = run(cos_theta, target)
    return loss
